# revision 6
# baseline (speedup 1.0000x reference)
"""KNN grouped-vector-attention pool kernel for 8 Trainium2 NeuronCores.

Strategy: shard queries M=16384 across 8 cores (2048 each). The context
feature table is sharded across cores (16384 rows each, fp16) and
reassembled on device with an HBM AllGather; each core then resolves its own
KNN gathers locally via indirect DMA and XBAR DMA-transposes into
channel-major layout. Relative positions (tiny) are pre-gathered on the
host. All per-core inputs are packed into one contiguous fp16 blob (~5.2MB)
so the host->device path pays a single transfer per core (per-array fixed
cost dominates this interconnect). Structured matrices (Sel / Ww1s / WpW1s)
are synthesized on device from tiny seeds. All matmuls run fp16 x fp16 with
fp32 PSUM accumulation; output returns as fp16.
"""
import sys
sys.path.insert(0, '/opt/trn_rl_repo')
import numpy as np

N_CORES = 8
M, N, K, C, G = 16384, 131072, 16, 128, 8
M_LOC = M // N_CORES          # 2048 queries per core
R_LOC = M_LOC * K             # 32768 gathered rows per core
N_LOC = N // N_CORES          # 16384 context rows uploaded per core
CHUNK = 512                   # rows per compute chunk (one PSUM bank)
GROUP = 16 * CHUNK            # 8192 rows per stacked group
N_GROUPS = R_LOC // GROUP     # 4
EPS_BN = 1e-5

# blob layout: (name, (partitions, cols)) packed row-major, fp16
_LAYOUT = [
    ("ctxslice", (C, N_LOC)),    # this core's context-feat rows, flat
    ("qfT", (C, M_LOC)),
    ("Wq", (C, C)), ("Wk", (C, C)), ("Wv", (C, C)),
    ("Wp2", (C, C)), ("W2bd", (C, C)),
    ("Ww1", (C, G)), ("P2W1", (C, G)),
    ("scal", (C, 9)),            # sq,bq,sk,bk,bv,sp1,bp1,sw1,bw1
    ("knn16", (C, R_LOC // C * 2)),  # [128,256] i32 KNN blocks, fp16 bits
    ("posT", (3, R_LOC)),
    ("Wp1", (3, C)),
]
_OFFS = {}
_NTOT = 0
for _nm, (_p, _c) in _LAYOUT:
    _OFFS[_nm] = _NTOT
    _NTOT += _p * _c

_compiled = None


def _build():
    from concourse import bacc, bass, mybir
    import concourse.tile as tile

    f32 = mybir.dt.float32
    f16 = mybir.dt.float16
    i32 = mybir.dt.int32
    AF = mybir.ActivationFunctionType
    OP = mybir.AluOpType

    nc = bacc.Bacc("TRN2", target_bir_lowering=False, debug=False,
                   num_devices=N_CORES)

    blob = nc.dram_tensor("blob", (_NTOT,), f16, kind="ExternalInput").ap()
    out_d = nc.dram_tensor("out", (C, M_LOC), f16, kind="ExternalOutput").ap()

    def view(nm):
        p, c = dict(_LAYOUT)[nm]
        off = _OFFS[nm]
        return blob[off:off + p * c].rearrange("(p c) -> p c", p=p)

    from contextlib import ExitStack
    est = ExitStack()
    with tile.TileContext(nc) as tc, est:
        dpool = est.enter_context(tc.tile_pool(name="dram", bufs=1, space="DRAM"))
        cpool = est.enter_context(tc.tile_pool(name="const", bufs=1))
        gtpool = est.enter_context(tc.tile_pool(name="gt", bufs=4))
        gpool = est.enter_context(tc.tile_pool(name="gath", bufs=2))
        vpool = est.enter_context(tc.tile_pool(name="valp", bufs=2))
        spool = est.enter_context(tc.tile_pool(name="work", bufs=2))
        opool = est.enter_context(tc.tile_pool(name="outp", bufs=1))
        ps = {}
        for nm, nb in [("kp", 2), ("px", 2), ("vp", 2), ("stk", 1)]:
            ps[nm] = est.enter_context(tc.tile_pool(name=nm, bufs=nb, space="PSUM"))

        # ---- AllGather the context-feature table in HBM --------------
        ib = dpool.tile([C, N_LOC], f16, tag="ib", name="ib")
        ob = dpool.tile([C, N_LOC * N_CORES], f16, tag="ob", name="ob",
                        addr_space="Shared")
        nc.gpsimd.dma_start(ib[:], view("ctxslice"))
        nc.gpsimd.collective_compute(
            "AllGather", OP.bypass,
            replica_groups=[list(range(N_CORES))],
            ins=[ib.opt()], outs=[ob.opt()])
        # reinterpret the gathered flat buffer as [N, C] row-major
        ctx2d = ob[:].rearrange("p (r c) -> (p r) c", c=C)

        # ---- constants into SBUF -------------------------------------
        ct = {}
        for nm in ("qfT", "Wq", "Wk", "Wv", "Wp2", "W2bd", "Ww1", "P2W1",
                   "scal", "knn16", "Wp1"):
            p, c = dict(_LAYOUT)[nm]
            ct[nm] = cpool.tile([p, c], f16, tag=f"c_{nm}", name=f"c_{nm}")
            nc.sync.dma_start(out=ct[nm][:], in_=view(nm))
        knn32 = ct["knn16"][:].bitcast(i32)          # [128, R_LOC/128] i32
        # fp16 scalars -> f32 working copy; per-scalar column APs
        scal32 = cpool.tile([C, 9], f32, tag="c_scal32", name="c_scal32")
        nc.vector.tensor_copy(out=scal32[:], in_=ct["scal"][:])
        for j, nm in enumerate(("sq", "bq", "sk", "bk", "bv", "sp1", "bp1",
                                "sw1", "bw1")):
            ct[nm] = scal32[:, j:j + 1]

        # ---- synthesize Sel / Ww1s / WpW1s on device -----------------
        # Sel[p, j] = 1 iff j // 16 == p  (i.e. 0 <= j - 16p <= 15)
        sel = cpool.tile([C, 16 * C], f16, tag="c_sel", name="c_sel")
        nc.gpsimd.memset(sel[:], 1.0)
        nc.gpsimd.affine_select(out=sel[:], in_=sel[:], compare_op=OP.is_ge,
                                fill=0.0, base=0, pattern=[[1, 16 * C]],
                                channel_multiplier=-16)
        nc.gpsimd.affine_select(out=sel[:], in_=sel[:], compare_op=OP.is_gt,
                                fill=0.0, base=16, pattern=[[-1, 16 * C]],
                                channel_multiplier=16)
        # Ww1s block i holds Ww1 at cols i*C + 8i .. +8 (rest zero)
        ww1s = cpool.tile([C, 16 * C], f16, tag="c_ww1s", name="c_ww1s")
        wpw1s = cpool.tile([C, 16 * C], f16, tag="c_wpw1s", name="c_wpw1s")
        nc.gpsimd.memset(ww1s[:], 0.0)
        nc.gpsimd.memset(wpw1s[:], 0.0)
        for i in range(16):
            c0 = i * C + 8 * i
            nc.vector.tensor_copy(out=ww1s[:, c0:c0 + 8], in_=ct["Ww1"][:])
            nc.vector.tensor_copy(out=wpw1s[:, c0:c0 + 8], in_=ct["P2W1"][:])

        # ---- qT = relu(bn(Wq.T @ qfT)) fp16 [C, M_LOC]; nqT = -qT -----
        qT = cpool.tile([C, M_LOC], f16, tag="c_qT", name="c_qT")
        nqT = cpool.tile([C, M_LOC], f16, tag="c_nqT", name="c_nqT")
        for t in range(M_LOC // CHUNK):
            q_ps = ps["kp"].tile([C, CHUNK], f32, tag="kp_t", name="q_ps")
            nc.tensor.matmul(out=q_ps[:], lhsT=ct["Wq"][:],
                             rhs=ct["qfT"][:, t * CHUNK:(t + 1) * CHUNK],
                             start=True, stop=True)
            nc.scalar.activation(out=qT[:, t * CHUNK:(t + 1) * CHUNK],
                                 in_=q_ps[:], func=AF.Relu,
                                 bias=ct["bq"], scale=ct["sq"])
            nc.scalar.activation(out=nqT[:, t * CHUNK:(t + 1) * CHUNK],
                                 in_=qT[:, t * CHUNK:(t + 1) * CHUNK],
                                 func=AF.Identity, scale=-1.0)

        outT = opool.tile([C, M_LOC], f32)

        for g in range(N_GROUPS):
            fT = gpool.tile([C, GROUP], f16, tag="fT")
            # gather + transpose this group's 8192 neighbor rows
            for blk in range(GROUP // C):
                gcol = g * (GROUP // C) + blk
                gt = gtpool.tile([C, C], f16, tag="gt")
                nc.gpsimd.indirect_dma_start(
                    out=gt[:], out_offset=None,
                    in_=ctx2d,
                    in_offset=bass.IndirectOffsetOnAxis(
                        ap=knn32[:, gcol:gcol + 1], axis=0))
                nc.sync.dma_start_transpose(
                    out=fT[:, blk * C:(blk + 1) * C], in_=gt[:])
            pT = gpool.tile([3, GROUP], f16, tag="pT")
            nc.sync.dma_start(out=pT[:],
                              in_=view("posT")[:, g * GROUP:(g + 1) * GROUP])
            valT = vpool.tile([C, GROUP], f32, tag="valp")
            stacked_ps = ps["stk"].tile([C, CHUNK], f32, tag="stk_t", name="stacked_ps")
            # -------- phase A: per chunk of 512 gathered rows ---------
            for i in range(16):
                ch = g * 16 + i              # global chunk id
                q0 = ch * 32                 # first query of chunk
                ctx = fT[:, i * CHUNK:(i + 1) * CHUNK]
                pos = pT[:, i * CHUNK:(i + 1) * CHUNK]
                # key = relu(bn(Wk.T @ ctx))
                k_ps = ps["kp"].tile([C, CHUNK], f32, tag="kp_t", name="k_ps")
                nc.tensor.matmul(out=k_ps[:], lhsT=ct["Wk"][:], rhs=ctx,
                                 start=True, stop=True)
                keyT = spool.tile([C, CHUNK], f16, tag="keyT")
                nc.scalar.activation(out=keyT[:], in_=k_ps[:], func=AF.Relu,
                                     bias=ct["bk"], scale=ct["sk"])
                # pebx = relu(bn(Wp1.T @ pos))
                pebx_ps = ps["px"].tile([C, CHUNK], f32, tag="px_t", name="pebx_ps")
                nc.tensor.matmul(out=pebx_ps[:], lhsT=ct["Wp1"][:], rhs=pos,
                                 start=True, stop=True)
                pebxT = spool.tile([C, CHUNK], f16, tag="pebxT")
                nc.scalar.activation(out=pebxT[:], in_=pebx_ps[:], func=AF.Relu,
                                     bias=ct["bp1"], scale=ct["sp1"])
                # val = Wv.T @ ctx + Wp2.T @ pebx (+ bv + bp2 via bias)
                v_ps = ps["vp"].tile([C, CHUNK], f32, tag="vp_t", name="v_ps")
                nc.tensor.matmul(out=v_ps[:], lhsT=ct["Wv"][:], rhs=ctx,
                                 start=True, stop=False)
                nc.tensor.matmul(out=v_ps[:], lhsT=ct["Wp2"][:], rhs=pebxT[:],
                                 start=False, stop=True)
                nc.scalar.activation(out=valT[:, i * CHUNK:(i + 1) * CHUNK],
                                     in_=v_ps[:], func=AF.Identity,
                                     bias=ct["bv"], scale=1.0)
                # w1 logits, stacked: Ww1.T @ (key - q + peb) with
                # peb folded via WpW1s = Wp2 @ Ww1s and -q via nqT
                q_rep = nqT[:, q0:q0 + 32].unsqueeze(2).to_broadcast([C, 32, K])
                nc.tensor.matmul(out=stacked_ps[:],
                                 lhsT=ww1s[:, i * C:(i + 1) * C],
                                 rhs=keyT[:], start=(i == 0), stop=False,
                                 skip_group_check=True)
                nc.tensor.matmul(out=stacked_ps[:],
                                 lhsT=wpw1s[:, i * C:(i + 1) * C],
                                 rhs=pebxT[:], start=False, stop=False,
                                 skip_group_check=True)
                nc.tensor.matmul(out=stacked_ps[:],
                                 lhsT=ww1s[:, i * C:(i + 1) * C],
                                 rhs=q_rep, start=False, stop=(i == 15),
                                 skip_group_check=True)
            # -------- group tail: bn/relu, mm2, softmax ---------------
            stk_bn = spool.tile([C, CHUNK], f16, tag="stkbn")
            nc.scalar.activation(out=stk_bn[:], in_=stacked_ps[:], func=AF.Relu,
                                 bias=ct["bw1"], scale=ct["sw1"])
            w2_ps = ps["px"].tile([C, CHUNK], f32, tag="px_t", name="w2_ps")
            nc.tensor.matmul(out=w2_ps[:], lhsT=ct["W2bd"][:], rhs=stk_bn[:],
                             start=True, stop=True)
            mx = spool.tile([C, 32], f32, tag="mx")
            nc.vector.tensor_reduce(
                out=mx[:], in_=w2_ps[:].rearrange("p (m k) -> p m k", k=K),
                axis=mybir.AxisListType.X, op=OP.max)
            sm = spool.tile([C, CHUNK], f32, tag="sm")
            nc.vector.tensor_tensor(
                out=sm[:].rearrange("p (m k) -> p m k", k=K),
                in0=w2_ps[:].rearrange("p (m k) -> p m k", k=K),
                in1=mx[:].unsqueeze(2).to_broadcast([C, 32, K]),
                op=OP.subtract)
            e_t = spool.tile([C, CHUNK], f32, tag="e")
            nc.scalar.activation(out=e_t[:], in_=sm[:], func=AF.Exp)
            s_t = spool.tile([C, 32], f32, tag="s")
            nc.vector.tensor_reduce(
                out=s_t[:], in_=e_t[:].rearrange("p (m k) -> p m k", k=K),
                axis=mybir.AxisListType.X, op=OP.add)
            rinv = spool.tile([C, 32], f32, tag="rinv")
            nc.vector.reciprocal(out=rinv[:], in_=s_t[:])
            wf32 = spool.tile([C, CHUNK], f32, tag="wf32")
            nc.vector.tensor_tensor(
                out=wf32[:].rearrange("p (m k) -> p m k", k=K),
                in0=e_t[:].rearrange("p (m k) -> p m k", k=K),
                in1=rinv[:].unsqueeze(2).to_broadcast([C, 32, K]),
                op=OP.mult)
            wfin = spool.tile([C, CHUNK], f16, tag="wfin")
            nc.scalar.activation(out=wfin[:], in_=wf32[:], func=AF.Identity)
            # -------- phase B: weighted sum per chunk -----------------
            for i in range(16):
                ch = g * 16 + i
                wrep_ps = ps["kp"].tile([C, CHUNK], f32, tag="kp_t", name="wrep_ps")
                nc.tensor.matmul(out=wrep_ps[:],
                                 lhsT=sel[:, i * C:(i + 1) * C],
                                 rhs=wfin[:], start=True, stop=True)
                prod = spool.tile([C, CHUNK], f32, tag="prod")
                nc.vector.tensor_tensor(out=prod[:],
                                        in0=valT[:, i * CHUNK:(i + 1) * CHUNK],
                                        in1=wrep_ps[:], op=OP.mult)
                nc.vector.tensor_reduce(
                    out=outT[:, ch * 32:(ch + 1) * 32],
                    in_=prod[:].rearrange("p (m k) -> p m k", k=K),
                    axis=mybir.AxisListType.X, op=OP.add)

        outT16 = opool.tile([C, M_LOC], f16, tag="out16", name="out16")
        nc.scalar.activation(out=outT16[:], in_=outT[:], func=AF.Identity)
        nc.sync.dma_start(out=out_d[:], in_=outT16[:])

    nc.compile()
    return nc


def _prep_inputs(inputs):
    """Host-side marshaling: shard context, gather positions, fp16 blob pack."""
    f = np.float32
    h = np.float16
    ctx_f = np.asarray(inputs["context_feat"], f)
    ctx_c = np.asarray(inputs["context_coord"], f)
    ctx16 = ctx_f.astype(h)                                    # [N, C]

    s = lambda g_: (np.asarray(g_, f) / np.sqrt(np.float32(1.0 + EPS_BN)))
    Wq = np.asarray(inputs["Wq"], f); Wk = np.asarray(inputs["Wk"], f)
    Wv = np.asarray(inputs["Wv"], f)
    Wp1 = np.asarray(inputs["Wp1"], f); Wp2 = np.asarray(inputs["Wp2"], f)
    Ww1 = np.asarray(inputs["Ww1"], f); Ww2 = np.asarray(inputs["Ww2"], f)

    sq = s(inputs["gq"]); bq = sq * inputs["bq"] + np.asarray(inputs["betaq"], f)
    sk = s(inputs["gk"]); bk = sk * inputs["bk"] + np.asarray(inputs["betak"], f)
    sp1 = s(inputs["gp1"])
    bp1 = sp1 * inputs["bp1"] + np.asarray(inputs["betap1"], f)
    bv = np.asarray(inputs["bv"], f) + np.asarray(inputs["bp2"], f)  # val bias
    # stacked bn for w1: row 8i+g ; fold bp2@Ww1 into bias
    sw1_g = s(inputs["gw1"])                                   # [G]
    bw1_g = (sw1_g * (np.asarray(inputs["bw1"], f)
                      + np.asarray(inputs["bp2"], f) @ Ww1)
             + np.asarray(inputs["betaw1"], f))                # [G]
    sw1 = np.tile(sw1_g, 16).astype(f)
    bw1 = np.tile(bw1_g, 16).astype(f)

    P2W1 = (Wp2 @ Ww1).astype(f)                               # [C, G]
    W2bd = np.zeros((C, C), f)
    for i in range(16):
        W2bd[8 * i:8 * i + 8, 8 * i:8 * i + 8] = Ww2

    scal = np.stack([sq, bq, sk, bk, bv, sp1, bp1, sw1, bw1], axis=1)  # [C,9]

    knn = np.asarray(inputs["knn_indexes"])
    knn = np.where(knn < 0, 0, knn).astype(np.int32)
    qf = np.asarray(inputs["query_feat"], f)
    qc = np.asarray(inputs["query_coord"], f)

    fixed = {"Wq": Wq, "Wk": Wk, "Wv": Wv, "Wp2": Wp2, "W2bd": W2bd,
             "Ww1": Ww1, "P2W1": P2W1, "scal": scal, "Wp1": Wp1}
    fixed16 = {nm: np.asarray(v, f).astype(h).ravel() for nm, v in fixed.items()}

    in_maps = []
    for c in range(N_CORES):
        sl = slice(c * M_LOC, (c + 1) * M_LOC)
        idx = knn[sl].reshape(-1)                        # [R_LOC] m*16+k order
        knn_t = idx.reshape(R_LOC // C, C).T.copy()      # [128, R_LOC/128] i32
        blob = np.empty(_NTOT, h)
        pieces = dict(fixed16)
        pieces["ctxslice"] = ctx16[c * N_LOC:(c + 1) * N_LOC].ravel()
        pieces["qfT"] = qf[sl].T.astype(h).ravel()
        pieces["knn16"] = knn_t.view(h).ravel()
        pieces["posT"] = (ctx_c[idx] - np.repeat(qc[sl], K, axis=0)) \
            .T.astype(h).ravel()
        for nm, (p_, c_) in _LAYOUT:
            off = _OFFS[nm]
            blob[off:off + p_ * c_] = pieces[nm]
        in_maps.append({"blob": blob})
    return in_maps


def kernel(**inputs):
    global _compiled
    from concourse.bass_utils import run_bass_kernel_spmd
    if _compiled is None:
        _compiled = _build()
    in_maps = _prep_inputs(inputs)
    res = run_bass_kernel_spmd(_compiled, in_maps, core_ids=list(range(N_CORES)))
    out = np.concatenate([res.results[c]["out"].T for c in range(N_CORES)], axis=0)
    return np.ascontiguousarray(out.astype(np.float32))
